# revision 38
# baseline (speedup 1.0000x reference)
"""Trainium2 Bass kernel for nn_AttentionLayer_1666447311232 (gnn_message_passing).

Reference computation per (b, l):
  ac[k,g,h] += self_att[g,h] (k=0 slot)
  sac = ac * (beta[g,h] + EPS)
  e = exp(sac)                  # max subtraction skipped: validated ~1.6e-6 rel
                                # err; max |sac| ~ 6.4 so exp never overflows
  acf[k,h] = sum_g gw[k,g] * e[k,g,h]
  acfn = acf / (S + EPS), S = sum_k acf  # acf >= 0 so abs is a no-op
  out[f,h] = sum_k nodes[k,f,h] * acfn[k,h]
            = (sum_k nodes[k,f,h] * acf[k,h]) * (1/(S+EPS))

Sharding: data-parallel over batch B=8 across the 8 NeuronCores.
Per-core layout: L on SBUF partitions (16 tiles of 128), (K,G,H)/(K,F,H) on the
free dim. ACT does exp; DVE runs the ac chain and half the nodes path; POOL
consumes DVE results (self-add, g-reduce part, nodes half, normalize) so it
never blocks DVE mid-tile.
"""

import contextlib

import numpy as np

B, L, K, G, H, F = 8, 2048, 32, 4, 16, 8
GH, FH = G * H, F * H
EPS = 1e-6
P = 128
NT = L // P  # 16 tiles per core
N_CORES = 8
CCHUNK = 4   # const DMA chunking (tiles per chunk)

AC_BUFS = 4
NODES_BUFS = 4
MID_BUFS = 4
SELF_ADD_DVE = False
ACFN_LATE = False

_NC_CACHE = {}


def _null():
    return contextlib.nullcontext()


def build_nc(reps=1):
    import concourse.tile as tile
    from concourse import bacc, mybir

    f32 = mybir.dt.float32
    Alu = mybir.AluOpType
    Act = mybir.ActivationFunctionType

    nc = bacc.Bacc()

    beta_d = nc.dram_tensor("beta", (L, GH), f32, kind="ExternalInput")
    sa_d = nc.dram_tensor("self_attention", (L, GH), f32, kind="ExternalInput")
    ac_d = nc.dram_tensor("attention_coefficients", (L, K, GH), f32, kind="ExternalInput")
    nodes_d = nc.dram_tensor("node_outputs", (L, K, FH), f32, kind="ExternalInput")
    gw_d = nc.dram_tensor("graph_weights", (L, K, G), f32, kind="ExternalInput")
    out_d = nc.dram_tensor("out", (L, FH), f32, kind="ExternalOutput")
    acf_d = nc.dram_tensor("acf", (L, K, H), f32, kind="ExternalOutput")

    beta_a, sa_a, ac_a, nodes_a, gw_a = (t[:] for t in (beta_d, sa_d, ac_d, nodes_d, gw_d))
    out_a, acf_a = out_d[:], acf_d[:]

    with tile.TileContext(nc) as tc:
        with (
            tc.tile_pool(name="acp", bufs=AC_BUFS) as acp,
            tc.tile_pool(name="nodesp", bufs=NODES_BUFS) as nodesp,
            tc.tile_pool(name="mid", bufs=MID_BUFS) as mid,
            tc.tile_pool(name="consts", bufs=1) as consts,
        ):
            eps_t = consts.tile([P, 1], f32, tag="eps")
            nc.vector.memset(eps_t, EPS)

            # small per-l tensors for the whole core: partition p holds
            # l = i*128 + p at free position i; chunked so tile 0 isn't blocked
            beta_all = consts.tile([P, NT, GH], f32, tag="beta_all")
            sa_all = consts.tile([P, NT, GH], f32, tag="sa_all")
            gw_all = consts.tile([P, NT, K, G], f32, tag="gw_all")
            beta_src = beta_a.rearrange("(i p) c -> p i c", p=P)
            sa_src = sa_a.rearrange("(i p) c -> p i c", p=P)
            gw_src = gw_a.rearrange("(i p) k g -> p i k g", p=P)

            def const_chunk(c0):
                sl = slice(c0, c0 + CCHUNK)
                nc.sync.dma_start(out=beta_all[:, sl], in_=beta_src[:, sl])
                nc.sync.dma_start(out=sa_all[:, sl], in_=sa_src[:, sl])
                nc.sync.dma_start(out=gw_all[:, sl], in_=gw_src[:, sl])
                # sfac = exp((beta+EPS)*sa): the k=0 self-attention term as a
                # multiplicative factor (exp(b*(ac+sa)) == exp(b*ac)*exp(b*sa))
                nc.vector.scalar_tensor_tensor(
                    out=sa_all[:, sl], in0=beta_all[:, sl], scalar=EPS,
                    in1=sa_all[:, sl], op0=Alu.add, op1=Alu.mult,
                )
                nc.scalar.activation(out=sa_all[:, sl], in_=sa_all[:, sl],
                                     func=Act.Exp)

            n = NT * reps
            ac_tiles = {}

            def fetch_ac(i):
                it = i % NT
                t = acp.tile([P, K, GH], f32, tag="ac")
                nc.sync.dma_start(out=t, in_=ac_a[it * P:(it + 1) * P])
                ac_tiles[i] = t

            fetch_ac(0)
            for i in range(n):
                it = i % NT
                l0 = it * P

                ac_t = ac_tiles.pop(i)
                nodes_t = nodesp.tile([P, K, FH], f32, tag="nodes")
                if i < NT and it % CCHUNK == 0:
                    const_chunk(it)
                if i + 1 < n:
                    fetch_ac(i + 1)
                nc.sync.dma_start(out=nodes_t, in_=nodes_a[l0:l0 + P])

                ac4 = ac_t[:].rearrange("p k (g h) -> p k g h", g=G)

                # (b) sac = (beta + EPS) * ac  (DVE, fused +EPS via STT)
                beta_b = beta_all[:, it].rearrange("p (g h) -> p g h", g=G) \
                    .unsqueeze(1).broadcast_to([P, K, G, H])
                nc.vector.scalar_tensor_tensor(
                    out=ac4, in0=beta_b, scalar=EPS, in1=ac4,
                    op0=Alu.add, op1=Alu.mult,
                )
                # (c) e = exp(sac) in place  (ACT)
                nc.scalar.activation(out=ac_t, in_=ac_t, func=Act.Exp)
                # (d) e *= gw broadcast over H  (DVE); then the k=0 self-att
                #     factor (same engine to avoid cross-engine WAW coupling)
                gw_b = gw_all[:, it].unsqueeze(3).broadcast_to([P, K, G, H])
                nc.vector.tensor_mul(out=ac4, in0=ac4, in1=gw_b)
                sfac = sa_all[:, it].rearrange("p (g h) -> p g h", g=G)
                nc.vector.tensor_mul(out=ac4[:, 0], in0=ac4[:, 0], in1=sfac)

                # (e) acf[k,h] = sum_g: pairwise adds; lv1b on POOL (consumer)
                acf_t = mid.tile([P, K, H], f32, tag="acf")
                nc.vector.tensor_add(out=ac4[:, :, 0], in0=ac4[:, :, 0], in1=ac4[:, :, 1])
                nc.gpsimd.tensor_add(out=ac4[:, :, 2], in0=ac4[:, :, 2], in1=ac4[:, :, 3])
                nc.vector.tensor_add(out=acf_t, in0=ac4[:, :, 0], in1=ac4[:, :, 2])

                # (f) S[h] = sum_k acf[k,h]  (DVE reduce, innermost of (h,k) view)
                s_t = mid.tile([P, H], f32, tag="s")
                nc.vector.reduce_sum(
                    out=s_t, in_=acf_t[:].transpose([0, 2, 1]),
                    axis=mybir.AxisListType.X,
                )
                # (g) Sinv = 1/(S+EPS)  (+EPS on ACT, reciprocal on DVE)
                nc.scalar.activation(out=s_t, in_=s_t,
                                     func=Act.Identity, bias=eps_t[:])
                nc.vector.reciprocal(out=s_t, in_=s_t)

                # (j) prod = nodes * acf (un-normalized; Sinv applied at the end)
                #     k-halves: DVE does [0:16], POOL does [16:32]
                nodes4 = nodes_t[:].rearrange("p k (f h) -> p k f h", f=F)
                acf_b = acf_t[:].unsqueeze(2).broadcast_to([P, K, F, H])
                nc.vector.tensor_mul(out=nodes4[:, 0:16], in0=nodes4[:, 0:16],
                                     in1=acf_b[:, 0:16])
                nc.gpsimd.tensor_mul(out=nodes4[:, 16:32], in0=nodes4[:, 16:32],
                                     in1=acf_b[:, 16:32])

                if not ACFN_LATE:
                    s_b = s_t[:].unsqueeze(1).broadcast_to([P, K, H])
                    nc.gpsimd.tensor_mul(out=acf_t, in0=acf_t, in1=s_b)
                    nc.sync.dma_start(out=acf_a[l0:l0 + P], in_=acf_t)

                # (k) per-half reduction trees: DVE on k[0:16], POOL on k[16:32]
                for half in (8, 4, 2, 1):
                    nc.vector.tensor_add(
                        out=nodes_t[:, 0:half],
                        in0=nodes_t[:, 0:half],
                        in1=nodes_t[:, half:2 * half],
                    )
                    nc.gpsimd.tensor_add(
                        out=nodes_t[:, 16:16 + half],
                        in0=nodes_t[:, 16:16 + half],
                        in1=nodes_t[:, 16 + half:16 + 2 * half],
                    )
                # (l) combine halves and scale by Sinv:
                #     out = (suma + sumb) * Sinv  via STT on DVE
                out_t = mid.tile([P, F, H], f32, tag="out")
                sinv_fb = s_t[:].unsqueeze(1).broadcast_to([P, F, H])
                nc.vector.scalar_tensor_tensor(
                    out=out_t, in0=nodes_t[:, 0].rearrange("p (f h) -> p f h", f=F),
                    scalar=1.0,
                    in1=nodes_t[:, 16].rearrange("p (f h) -> p f h", f=F),
                    op0=Alu.mult, op1=Alu.add,
                )
                nc.vector.tensor_mul(out=out_t, in0=out_t, in1=sinv_fb)
                # (m) store out
                nc.sync.dma_start(out=out_a[l0:l0 + P], in_=out_t[:].rearrange("p f h -> p (f h)"))

                if ACFN_LATE:
                    # (h) acfn = acf * Sinv broadcast over K  (POOL)
                    s_b = s_t[:].unsqueeze(1).broadcast_to([P, K, H])
                    nc.gpsimd.tensor_mul(out=acf_t, in0=acf_t, in1=s_b)
                    # (i) store acfn
                    nc.sync.dma_start(out=acf_a[l0:l0 + P], in_=acf_t)

    nc.finalize()
    return nc


def _get_nc(reps=1):
    if reps not in _NC_CACHE:
        _NC_CACHE[reps] = build_nc(reps)
    return _NC_CACHE[reps]


def _run(inputs, reps=1, **kw):
    from concourse.bass_utils import run_bass_kernel_spmd

    nc = _get_nc(reps)
    in_maps = []
    for b in range(N_CORES):
        in_maps.append({
            "beta": np.ascontiguousarray(inputs["beta"][b]),
            "self_attention": np.ascontiguousarray(inputs["self_attention"][b]),
            "attention_coefficients": np.ascontiguousarray(
                inputs["attention_coefficients"][b]).reshape(L, K, GH),
            "node_outputs": np.ascontiguousarray(
                inputs["node_outputs"][b]).reshape(L, K, FH),
            "graph_weights": np.ascontiguousarray(inputs["graph_weights"][b]),
        })
    res = run_bass_kernel_spmd(nc, in_maps, core_ids=list(range(N_CORES)), **kw)
    out = np.stack([r["out"] for r in res.results])
    acf = np.stack([r["acf"] for r in res.results]).reshape(B, L, K, H)
    return (out, acf), res


def kernel(**inputs):
    (out, acf), _ = _run(inputs)
    return out, acf
